# revision 1
# baseline (speedup 1.0000x reference)
"""GQA causal attention block (RMSNorm+RoPE+gain, flash-style) on 8 Trainium2 cores.

Problem: nn_Attention (B=2, S=2048, D=1024, H=16, KVH=4, HD=64), fp32.

Sharding: core c = (b, g) with b = c//4 (batch), g = c%4 (kv-head group).
Each core computes q-heads 4g..4g+3 and kv-head g for batch b, runs causal
attention for its 4 heads, and produces the partial wo product
  part_c = y_c @ wo[:, 256g:256g+256].T   in [2048, 1024].
The host sums the 4 partials per batch (tensor-parallel all-reduce done on
host during unsharding).

All matmuls run as float32r (TF32-like, 1 cycle/row at N>=256).

Device layout notes:
 - x is pre-transposed on host to xT [1024, 2048] so projections contract
   over D on partitions.
 - Fused qkv projection: rhs = [wqT | wkT | wvT] [1024, 384] -> psum [128, 384]
   per 128-row s-tile (q: 0:256, k: 256:320, v: 320:384).
 - RMSNorm stats + normalize + RoPE in natural [s, (h hd)] layout (DVE),
   with gain/sqrt(hd) folded into the per-head normalize scalars.
 - q/k transposed per s-tile on PE to qT/kT [hd, s]; v kept natural with an
   appended ones column (v1) so the attention out-matmul also produces the
   softmax denominator.
 - Scores computed transposed: scoresT[sk, sq] = kT_tile.T @ qT_chunk; exp on
   ACT (no max subtraction needed: |q|,|k| = sqrt(hd) after RMSNorm, so
   |score| <= 8); causal masking via per-diagonal-block multiplicative mask +
   partial-column matmuls for fully-masked left regions.
 - out_head[sq, 65] accumulated over sk-tiles: lhsT = v1, rhs = attnT.
   Column 64 = denominator. Normalization via DVE reciprocal + PE outer
   product broadcast + DVE multiply, written into packed y tiles [128, 2048]
   (two heads per tile) feeding the wo matmul with K=128.
"""

import os
import sys

sys.path.insert(0, "/opt/trn_rl_repo")

import numpy as np
import concourse.bass as bass
import concourse.mybir as mybir
import concourse.tile as tile
from concourse.bass_utils import run_bass_kernel_spmd

F32 = mybir.dt.float32
F32R = mybir.dt.float32r
AL = mybir.AluOpType
AF = mybir.ActivationFunctionType

B, S, D = 2, 2048, 1024
H, KVH, HD = 16, 4, 64
G = H // KVH          # q heads per core (= per kv head)
NC = 8
ST = 128              # s-tile rows
NST = S // ST         # 16
KT = 128              # contraction tile
NKT = D // KT         # 8
SQC = 512             # sq chunk width in attention
NSQC = S // SQC       # 4
ROPE_BASE = 10000.0
EPS = float(np.finfo(np.float32).eps)

LAST_EXEC_NS = None

_counter = [0]


def _split_waits(nc, cap=1):
    """Walrus in this toolchain rejects >1 sync wait per instruction; hoist
    extras onto same-engine NoOps."""
    n = 0
    for f in nc.m.functions:
        for blk in f.blocks:
            out = []
            for inst in blk.instructions:
                si = inst.sync_info
                if si is not None and si.on_wait and len(si.on_wait) > cap:
                    waits = list(si.on_wait)
                    extra, keep = waits[:-cap], waits[-cap:]
                    for w in extra:
                        _counter[0] += 1
                        out.append(
                            mybir.InstNoOp(
                                name=f"WSPLIT-{_counter[0]}",
                                engine=inst.engine,
                                ins=[],
                                outs=[],
                                sync_info=mybir.SyncInfo(on_wait=[w], on_update=[]),
                            )
                        )
                    inst.sync_info = mybir.SyncInfo(
                        on_wait=keep, on_update=list(si.on_update)
                    )
                    n += 1
                out.append(inst)
            blk.instructions[:] = out
    return n


def build_nc(reps=1):
    nc = bass.Bass("TRN2", target_bir_lowering=False, debug=False, num_devices=NC)

    xt_d = nc.dram_tensor("xt", [D, S], F32R, kind="ExternalInput").ap()
    wt_d = nc.dram_tensor("wt", [D, 384], F32R, kind="ExternalInput").ap()
    wot_d = nc.dram_tensor("wot", [G * HD, D], F32R, kind="ExternalInput").ap()
    cosd_d = nc.dram_tensor("cosd", [ST, NST * HD], F32R, kind="ExternalInput").ap()
    sind_d = nc.dram_tensor("sind", [ST, NST * HD], F32R, kind="ExternalInput").ap()
    gains_d = nc.dram_tensor("gains", [ST, 8], F32, kind="ExternalInput").ap()
    mask_d = nc.dram_tensor("mask", [ST, ST], F32R, kind="ExternalInput").ap()
    ident_d = nc.dram_tensor("ident", [ST, ST], F32R, kind="ExternalInput").ap()
    ones_d = nc.dram_tensor("onesr", [1, HD], F32R, kind="ExternalInput").ap()
    eps_d = nc.dram_tensor("epsc", [ST, 1], F32, kind="ExternalInput").ap()
    part_d = nc.dram_tensor("part", [S, D], F32, kind="ExternalOutput").ap()

    NH5 = G + 1  # 4 q heads + 1 k head share norm/rope
    CT = SQC // ST  # s-tiles per chunk (4)

    with tile.TileContext(nc) as tc:
        with (
            nc.allow_low_precision(reason="fp32r matmul inputs"),
            tc.tile_pool(name="persist", bufs=1) as pp,
            tc.tile_pool(name="work", bufs=2) as p1w,
            tc.tile_pool(name="xpool", bufs=2) as px,
            tc.tile_pool(name="attn", bufs=4) as p2,
            tc.tile_pool(name="attns", bufs=3) as p2s,
            tc.tile_pool(name="ps_mm", bufs=2, space="PSUM") as ps_mm,
            tc.tile_pool(name="ps_tr", bufs=2, space="PSUM") as ps_tr,
            tc.tile_pool(name="ps_s", bufs=2, space="PSUM") as ps_s,
            tc.tile_pool(name="ps_o", bufs=2, space="PSUM") as ps_o,
        ):
            # persistent tiles (constants + per-chunk activations)
            qTc = [
                [pp.tile([HD, SQC], F32R, tag=f"qT{h}_{qc}", name=f"qT{h}_{qc}")
                 for qc in range(NSQC)]
                for h in range(G)
            ]
            kTc = [pp.tile([HD, SQC], F32R, tag=f"kT{qc}", name=f"kT{qc}")
                   for qc in range(NSQC)]
            v1c = [pp.tile([ST, CT * (HD + 1)], F32R, tag=f"v1{qc}", name=f"v1c{qc}")
                   for qc in range(NSQC)]
            ypc = [
                [pp.tile([ST, SQC], F32R, tag=f"yp{t}_{qc}", name=f"yp{t}_{qc}")
                 for qc in range(NSQC)]
                for t in range(2)
            ]
            wotp = [pp.tile([ST, D], F32R, tag=f"wot{t}", name=f"wotp{t}")
                    for t in range(2)]
            cosd = pp.tile([ST, NST * HD], F32R, tag="cosd")
            sind = pp.tile([ST, NST * HD], F32R, tag="sind")
            gains = pp.tile([ST, 8], F32, tag="gains")
            maskt = pp.tile([ST, ST], F32R, tag="mask")
            ident = pp.tile([ST, ST], F32R, tag="ident")
            onest = pp.tile([1, HD], F32R, tag="ones")
            epst = pp.tile([ST, 1], F32, tag="eps")
            wts = [p1w.tile([KT, 384], F32R, tag=f"wt{k}", name=f"wts{k}", bufs=1)
                   for k in range(NKT)]

            for k in range(NKT):
                nc.sync.dma_start(out=wts[k][:], in_=wt_d[k * KT:(k + 1) * KT, :])
            nc.sync.dma_start(out=gains[:], in_=gains_d[:])
            nc.sync.dma_start(out=epst[:], in_=eps_d[:])
            nc.sync.dma_start(out=cosd[:], in_=cosd_d[:])
            nc.sync.dma_start(out=sind[:], in_=sind_d[:])
            nc.sync.dma_start(out=ident[:], in_=ident_d[:])
            nc.sync.dma_start(out=maskt[:], in_=mask_d[:])
            nc.sync.dma_start(out=onest[:], in_=ones_d[:])
            for t in range(2):
                nc.sync.dma_start(out=wotp[t][:], in_=wot_d[t * ST:(t + 1) * ST, :])

            for rep in range(reps):
                xtc_map = {}

                def emit_xtc(qc):
                    xtc = [
                        px.tile([KT, SQC], F32R, tag=f"xt{k}", name=f"xtc{k}_{qc}")
                        for k in range(NKT)
                    ]
                    for k in range(NKT):
                        nc.sync.dma_start(
                            out=xtc[k][:],
                            in_=xt_d[k * KT:(k + 1) * KT, qc * SQC:(qc + 1) * SQC],
                        )
                    xtc_map[qc] = xtc
                    # ones column of v1 for this chunk
                    v1g = v1c[qc][:].rearrange("p (m c) -> p m c", c=HD + 1)[:, :, HD:HD + 1]
                    src_ = gains[:, 0:1].unsqueeze(1).broadcast_to([ST, CT, 1])
                    nc.scalar.activation(v1g, src_, AF.Copy, bias=1.0, scale=0.0)

                def emit_p1_stile(qc, mm):
                    m = qc * CT + mm
                    xtc = xtc_map[qc]
                    ps = ps_mm.tile([ST, 384], F32, tag="proj", name="ps")
                    for k in range(NKT):
                        nc.tensor.matmul(
                            ps[:],
                            xtc[k][:, mm * ST:(mm + 1) * ST],
                            wts[k][:],
                            start=(k == 0),
                            stop=(k == NKT - 1),
                        )
                    # v evacuation (raw projection)
                    nc.scalar.activation(
                        v1c[qc][:, mm * (HD + 1):mm * (HD + 1) + HD],
                        ps[:, 320:384],
                        AF.Copy,
                    )
                    # rms stats: ACT Square (psum->sbuf), DVE segmented reduce
                    sq = p1w.tile([ST, 320], F32, tag="sq")
                    nc.scalar.activation(sq[:], ps[:, 0:320], AF.Square)
                    ss = p1w.tile([ST, 8], F32, tag="ss")
                    nc.vector.tensor_reduce(
                        ss[:, 0:NH5],
                        sq[:].rearrange("p (h d) -> p h d", d=HD),
                        axis=mybir.AxisListType.X,
                        op=AL.add,
                    )
                    # rstd = exp(-0.5*ln(ms+eps)); ln/exp share ACT table with
                    # Exp/Copy/Square -> no table reloads
                    lg = p1w.tile([ST, 8], F32, tag="lg")
                    nc.scalar.activation(
                        lg[:, 0:NH5], ss[:, 0:NH5], AF.Ln,
                        bias=epst[:, 0:1], scale=1.0 / HD,
                    )
                    rr = p1w.tile([ST, 8], F32, tag="rr")
                    nc.scalar.activation(rr[:, 0:NH5], lg[:, 0:NH5], AF.Exp, scale=-0.5)
                    rg = p1w.tile([ST, 8], F32, tag="rg")
                    nc.vector.tensor_tensor(
                        rg[:, 0:NH5], rr[:, 0:NH5], gains[:, 0:NH5], AL.mult
                    )
                    # rope on raw projections (normalize commutes with rope)
                    ps3 = ps[:, 0:320].rearrange("p (h d) -> p h d", d=HD)
                    cosm = cosd[:, m * HD:(m + 1) * HD]
                    sinm = sind[:, m * HD:(m + 1) * HD]
                    tcc = p1w.tile([ST, 320], F32, tag="tcc")
                    nc.vector.tensor_tensor(
                        tcc[:].rearrange("p (h d) -> p h d", d=HD),
                        ps3,
                        cosm.unsqueeze(1).broadcast_to([ST, NH5, HD]),
                        AL.mult,
                    )
                    tss = p1w.tile([ST, 320], F32, tag="tss")
                    tss3 = tss[:].rearrange("p (h d) -> p h d", d=HD)
                    HH = HD // 2
                    nc.vector.tensor_tensor(
                        tss3[:, :, 0:HH],
                        ps3[:, :, HH:HD],
                        sinm[:, 0:HH].unsqueeze(1).broadcast_to([ST, NH5, HH]),
                        AL.mult,
                    )
                    nc.vector.tensor_tensor(
                        tss3[:, :, HH:HD],
                        ps3[:, :, 0:HH],
                        sinm[:, HH:HD].unsqueeze(1).broadcast_to([ST, NH5, HH]),
                        AL.mult,
                    )
                    qkrr = p1w.tile([ST, 320], F32, tag="qkrr")
                    nc.gpsimd.tensor_tensor(qkrr[:], tcc[:], tss[:], AL.add)
                    # normalize q,k with folded gain/scale (rstd*gain bcast)
                    qkr = p1w.tile([ST, 320], F32R, tag="qkr")
                    nc.vector.tensor_tensor(
                        qkr[:].rearrange("p (h d) -> p h d", d=HD),
                        qkrr[:].rearrange("p (h d) -> p h d", d=HD),
                        rg[:, 0:NH5].unsqueeze(2).broadcast_to([ST, NH5, HD]),
                        AL.mult,
                    )
                    # transpose q (2x128 cols) and k (64 cols) to T layout
                    for t in range(2):
                        pt = ps_tr.tile([ST, ST], F32R, tag="tr", name="pt",
                                        padded_shape=[ST, SQC])
                        nc.tensor.transpose(pt[:], qkr[:, t * ST:(t + 1) * ST], ident[:])
                        nc.vector.tensor_copy(
                            qTc[2 * t][qc][:, mm * ST:(mm + 1) * ST], pt[0:HD, :]
                        )
                        nc.vector.tensor_copy(
                            qTc[2 * t + 1][qc][:, mm * ST:(mm + 1) * ST],
                            pt[HD:ST, :],
                        )
                    ptk = ps_tr.tile([HD, ST], F32R, tag="tr", name="ptk",
                                     padded_shape=[ST, SQC])
                    nc.tensor.transpose(ptk[:], qkr[:, 256:320], ident[:])
                    nc.scalar.activation(
                        kTc[qc][:, mm * ST:(mm + 1) * ST], ptk[:], AF.Copy
                    )

                def emit_attn_head(qc, h):
                    nsk = (qc + 1) * CT
                    po = ps_o.tile([HD + 1, SQC], F32, tag="po", name="po")
                    for sk in range(nsk):
                        skc, skm = sk // CT, sk % CT
                        pscr = ps_s.tile([ST, SQC], F32, tag="pscr", name="pscr")
                        nc.tensor.matmul(
                            pscr[:],
                            kTc[skc][:, skm * ST:(skm + 1) * ST],
                            qTc[h][qc][:],
                            start=True,
                            stop=True,
                        )
                        at = p2.tile([ST, SQC], F32R, tag="attn", name="at")
                        dj = sk - qc * CT
                        cb = dj * ST if dj >= 0 else 0
                        nc.scalar.activation(at[:, cb:SQC], pscr[:, cb:SQC], AF.Exp)
                        if dj >= 0:
                            nc.gpsimd.tensor_tensor(
                                at[:, cb:cb + ST], at[:, cb:cb + ST],
                                maskt[:], AL.mult,
                            )
                        nc.tensor.matmul(
                            po[:, cb:SQC],
                            v1c[skc][:, skm * (HD + 1):(skm + 1) * (HD + 1)],
                            at[:, cb:SQC],
                            start=(sk == 0),
                            stop=(sk == nsk - 1),
                        )
                    recf = p2s.tile([1, SQC], F32, tag="recf", name="recf")
                    nc.vector.reciprocal(recf[:], po[HD:HD + 1, :])
                    rec = p2s.tile([1, SQC], F32R, tag="rec", name="rec")
                    nc.vector.tensor_copy(rec[:], recf[:])
                    pb = ps_tr.tile([HD, SQC], F32, tag="tr", name="pb",
                                    padded_shape=[ST, SQC])
                    nc.tensor.matmul(pb[:], onest[:], rec[:], start=True, stop=True)
                    rb = p2s.tile([HD, SQC], F32, tag="rb", name="rb")
                    nc.vector.tensor_copy(rb[:], pb[:])
                    t, r = h // 2, (h % 2) * HD
                    nc.vector.tensor_tensor(
                        ypc[t][qc][r:r + HD, :], po[0:HD, :], rb[:], AL.mult
                    )

                def emit_p3_stile(qc, mm):
                    m = qc * CT + mm
                    for nch in range(2):
                        pw = ps_tr.tile([ST, SQC], F32, tag="tr", name="pw",
                                        padded_shape=[ST, SQC])
                        for t in range(2):
                            nc.tensor.matmul(
                                pw[:],
                                ypc[t][qc][:, mm * ST:(mm + 1) * ST],
                                wotp[t][:, nch * SQC:(nch + 1) * SQC],
                                start=(t == 0),
                                stop=(t == 1),
                            )
                        ob = p2.tile([ST, SQC], F32, tag="ob", name="ob")
                        nc.vector.tensor_copy(ob[:], pw[:])
                        nc.gpsimd.dma_start(
                            out=part_d[
                                m * ST:(m + 1) * ST,
                                nch * SQC:(nch + 1) * SQC,
                            ],
                            in_=ob[:],
                        )

                # software-pipelined emission: phase1(qc+1) s-tiles interleave
                # with attention(qc) heads; phase3(qc-1) rides along too.
                emit_xtc(0)
                for mm in range(CT):
                    emit_p1_stile(0, mm)
                for qc in range(NSQC):
                    if qc + 1 < NSQC:
                        emit_xtc(qc + 1)
                    for h in range(G):
                        emit_attn_head(qc, h)
                        if qc + 1 < NSQC:
                            emit_p1_stile(qc + 1, h)
                    for mm in range(CT):
                        emit_p3_stile(qc, mm)
    return nc


def _host_tables():
    inv_freq = 1.0 / (ROPE_BASE ** (np.arange(0, HD, 2, dtype=np.float32) / HD))
    pos = np.arange(S, dtype=np.float32)
    freqs = np.outer(pos, inv_freq)  # [S, 32]
    cos = np.cos(freqs).astype(np.float32)
    sin = np.sin(freqs).astype(np.float32)
    cosdup = np.concatenate([cos, cos], axis=1)        # [S, 64]
    sindup = np.concatenate([sin, -sin], axis=1)       # [S, 64]
    # rearrange [S, 64] -> [128, 16*64] with [p, m*64+j] = v[m*128+p, j]
    def arr(v):
        return np.ascontiguousarray(
            v.reshape(NST, ST, HD).transpose(1, 0, 2).reshape(ST, NST * HD)
        )
    mask = np.tril(np.ones((ST, ST), np.float32)).T  # mask[i,j] = 1 iff i<=j
    return arr(cosdup), arr(sindup), mask


_NC_CACHE = None


def _get_nc():
    global _NC_CACHE
    if _NC_CACHE is None:
        nc = build_nc()
        _split_waits(nc, cap=1)
        _NC_CACHE = nc
    return _NC_CACHE


def make_in_maps(x, wq, wk, wv, wo, q_gain):
    x = np.asarray(x, np.float32)
    wq, wk, wv, wo = (np.asarray(a, np.float32) for a in (wq, wk, wv, wo))
    q_gain = np.asarray(q_gain, np.float32)
    cosd, sind, mask = _host_tables()
    ident = np.eye(ST, dtype=np.float32)
    onesr = np.ones((1, HD), np.float32)
    epsc = np.full((ST, 1), EPS, np.float32)
    in_maps = []
    for c in range(NC):
        b, g = c // KVH, c % KVH
        xT = np.ascontiguousarray(x[b].T)  # [D, S]
        wq_c = wq[256 * g:256 * (g + 1), :]
        wk_c = wk[HD * g:HD * (g + 1), :]
        wv_c = wv[HD * g:HD * (g + 1), :]
        wt = np.ascontiguousarray(
            np.concatenate([wq_c.T, wk_c.T, wv_c.T], axis=1)
        )  # [D, 384]
        wot = np.ascontiguousarray(wo[:, 256 * g:256 * (g + 1)].T)  # [256, D]
        gains = np.zeros((ST, 8), np.float32)
        gains[:, 0:G] = q_gain[G * g:G * (g + 1)][None, :] / np.sqrt(HD)
        gains[:, G] = 1.0
        in_maps.append(
            dict(
                xt=xT, wt=wt, wot=wot, cosd=cosd, sind=sind, gains=gains,
                mask=mask, ident=ident, onesr=onesr, epsc=epsc,
            )
        )
    return in_maps


def kernel(x, wq, wk, wv, wo, q_gain):
    global LAST_EXEC_NS
    nc = _get_nc()
    in_maps = make_in_maps(x, wq, wk, wv, wo, q_gain)
    trace = os.environ.get("BASS_KERNEL_TRACE", "") == "1"
    r = run_bass_kernel_spmd(nc, in_maps, list(range(NC)), trace=trace)
    LAST_EXEC_NS = r.exec_time_ns
    parts = [r.results[c]["part"] for c in range(NC)]
    out = np.stack(
        [sum(parts[0:KVH]), sum(parts[KVH:2 * KVH])], axis=0
    ).astype(np.float32)
    return out

